# revision 23
# baseline (speedup 1.0000x reference)
"""Trainium2 Bass kernel for nn_AttentionBlock (B=16, C=512, H=W=32, 8 heads, d=64).

Sharding: data-parallel over batch — 2 batches per core on 8 cores, no collectives.

Per-core program (per batch):
  proj:    h[tok, head, 193] = x_b^T @ WpT   (lhsT = x_b native [C, S] layout;
           per-head layout is [q(64) | k(64) | v(64) | ones(1)] so PV can use
           lhsT = [V' | 1] as a single AP)
  pixnorm: token-major sumsq over each 64-group; SCALE^2 folded into q's factor
  q/k PE-transposed (head-pairs packed to 128 partitions) -> feature-major
  scores:  St[j, i] = k'^T q'  (row-packed head pairs, K=64)
  softmax: exp only (|logit| <= 8 by Cauchy-Schwarz after pixnorm -> no max sub)
  PV:      lhsT = [V' | 1] (M=65) -> res^T (feature-major) with Z fused in row 64
  norm:    recip(Z) + DMA partition-broadcast + elementwise mul
  outproj: out2^T = (WoT as lhsT) @ res^T (feature-major) + residual add with x_b
All matmuls run as float32r (FP22, full PE rate).
"""

import os
import sys

for _p in ("/opt/trn_rl_repo", "/root/.axon_site/_ro/trn_rl_repo"):
    if os.path.isdir(_p) and _p not in sys.path:
        sys.path.insert(0, _p)

import numpy as np

import concourse.bass as bass
from concourse import bacc
import concourse.tile as tile
from concourse import mybir
from concourse.bass import ts
from concourse.masks import make_identity

F32 = mybir.dt.float32
F32R = mybir.dt.float32r

B_LOCAL = 2      # batches per core
C = 512          # channels
S = 1024         # tokens (32*32)
NH = 8           # heads
D = 64           # head dim
O3 = 3 * C       # 1536
P = 128
MT = S // P      # 8 token tiles
KT = C // P      # 4 channel tiles
HS = 3 * D + 1   # per-head stride in h_sb: q,k,v,ones = 193
EPS = 1e-8

Alu = mybir.AluOpType
Act = mybir.ActivationFunctionType


def _proj_segments():
    """Split per-head 192-col blocks of the 1536-wide proj output at 512-col
    psum-tile boundaries. Returns {n: [(src_lo, head, within_lo, length)]}."""
    segs = {0: [], 1: [], 2: []}
    for head in range(NH):
        a = head * 192
        hi = a + 192
        while a < hi:
            n = a // 512
            b = min(hi, (n + 1) * 512)
            segs[n].append((a - n * 512, head, a - head * 192, b - a))
            a = b
    return segs


def build_kernel(nc):
    x = nc.dram_tensor("x", [B_LOCAL, C, S], F32, kind="ExternalInput")
    wpT = nc.dram_tensor("wpT", [C, O3], F32, kind="ExternalInput")
    woT = nc.dram_tensor("woT", [C, C], F32, kind="ExternalInput")
    y = nc.dram_tensor("y", [B_LOCAL, C, S], F32, kind="ExternalOutput")

    from contextlib import ExitStack

    with tile.TileContext(nc) as tc, ExitStack() as ctx:
        consts = ctx.enter_context(tc.tile_pool(name="consts", bufs=1))
        xpool = ctx.enter_context(tc.tile_pool(name="xpool", bufs=1))
        hpool = ctx.enter_context(tc.tile_pool(name="hpool", bufs=1))
        spool = ctx.enter_context(tc.tile_pool(name="spool", bufs=1))
        qkpool = ctx.enter_context(tc.tile_pool(name="qkpool", bufs=1))
        etpool = ctx.enter_context(tc.tile_pool(name="etpool", bufs=2))
        respool = ctx.enter_context(tc.tile_pool(name="respool", bufs=1))
        zpool = ctx.enter_context(tc.tile_pool(name="zpool", bufs=1))
        ypool = ctx.enter_context(tc.tile_pool(name="ypool", bufs=2))
        smpool = ctx.enter_context(tc.tile_pool(name="smpool", bufs=2))

        pw = ctx.enter_context(tc.tile_pool(name="pw", bufs=2, space="PSUM"))
        pst = ctx.enter_context(tc.tile_pool(name="pst", bufs=2, space="PSUM"))
        pres = ctx.enter_context(tc.tile_pool(name="pres", bufs=1, space="PSUM"))
        dpool = ctx.enter_context(tc.tile_pool(name="dpool", bufs=2, space="DRAM"))

        # weights + constants, loaded once
        wp_sb = consts.tile([P, KT, O3], F32R)
        nc.sync.dma_start(out=wp_sb, in_=wpT.rearrange("(kt p) o -> p kt o", p=P).bitcast(F32R))
        wo_sb = consts.tile([P, KT, C], F32R)
        nc.sync.dma_start(out=wo_sb, in_=woT.rearrange("(kt p) o -> p kt o", p=P).bitcast(F32R))
        identity = consts.tile([P, P], F32)
        make_identity(nc, identity)
        eps_q = consts.tile([P, 1], F32)
        nc.vector.memset(eps_q, float(D * EPS))
        eps_kv = consts.tile([P, 1], F32)
        nc.vector.memset(eps_kv, float(EPS))
        ones_sb = consts.tile([P, MT * NH], F32)
        nc.vector.memset(ones_sb, 1.0)

        segs = _proj_segments()

        for b in range(B_LOCAL):
            xb = xpool.tile([P, KT, S], F32R, name=f"xb{b}", tag="xb")
            nc.sync.dma_start(out=xb, in_=x[b].rearrange("(kt p) s -> p kt s", p=P).bitcast(F32R))

            h_sb = hpool.tile([P, MT, NH, HS], F32R, name=f"h{b}", tag="h")
            nc.sync.dma_start(
                out=h_sb[:, :, :, 192:193],
                in_=ones_sb.rearrange("p (m h) -> p m h", h=NH).unsqueeze(3).bitcast(F32R),
            )

            # ---- projection + pixnorm (token-major) ----
            for m in range(MT):
                for n in range(O3 // 512):
                    ps = pw.tile([P, 512], F32, name="ps", tag="pw")
                    for k in range(KT):
                        nc.tensor.matmul(
                            ps,
                            lhsT=xb[:, k, ts(m, P)],
                            rhs=wp_sb[:, k, ts(n, 512)],
                            start=(k == 0),
                            stop=(k == KT - 1),
                        )
                    for src_lo, head, w_lo, ln in segs[n]:
                        nc.vector.tensor_copy(
                            h_sb[:, m, head, w_lo : w_lo + ln],
                            ps[:, src_lo : src_lo + ln],
                        )

                # sum of squares per 64-group: [P, 24]
                sq = spool.tile([P, 24 * D], F32, name="sq", tag="sq")
                hqkv = h_sb[:, m, :, 0:192].rearrange("p h (t d) -> p h t d", d=D)
                nc.vector.scalar_tensor_tensor(
                    out=sq.rearrange("p (h t d) -> p h t d", t=3, d=D),
                    in0=hqkv,
                    scalar=1.0,
                    in1=hqkv,
                    op0=Alu.bypass,
                    op1=Alu.mult,
                )
                ss = smpool.tile([P, 3 * NH], F32, name="ss", tag="ss")
                nc.vector.reduce_sum(
                    out=ss,
                    in_=sq.rearrange("p (g d) -> p g d", d=D),
                    axis=mybir.AxisListType.X,
                )
                # factors: fac = 1/sqrt(scale*ss + bias)
                fac = smpool.tile([P, 3 * NH], F32, name="fac", tag="fac")
                ss3 = ss.rearrange("p (h t) -> p h t", t=3)
                fac3 = fac.rearrange("p (h t) -> p h t", t=3)
                # q: 1/sqrt(sumsq + 64*eps)  (SCALE^2 = 1/8 folded in)
                nc.scalar.activation(
                    out=fac3[:, :, 0:1], in_=ss3[:, :, 0:1], func=Act.Sqrt,
                    bias=eps_q, scale=1.0,
                )
                # k, v: 1/sqrt(sumsq/64 + eps)
                nc.scalar.activation(
                    out=fac3[:, :, 1:3], in_=ss3[:, :, 1:3], func=Act.Sqrt,
                    bias=eps_kv, scale=1.0 / D,
                )
                nc.vector.reciprocal(out=fac, in_=fac)

                # apply factors in place (broadcast over d)
                hqk = h_sb[:, m, :, 0:128].rearrange("p h (t d) -> p h t d", d=D)
                fqk = fac3[:, :, 0:2].unsqueeze(3).broadcast_to([P, NH, 2, D])
                nc.vector.tensor_mul(out=hqk, in0=hqk, in1=fqk)
                hv = h_sb[:, m, :, 128:192]
                fv = fac3[:, :, 2:3].broadcast_to([P, NH, D])
                nc.vector.tensor_mul(out=hv, in0=hv, in1=fv)

            # ---- attention, by head pair ----
            resn = respool.tile([P, KT, S], F32R, name=f"resn{b}", tag="resn")
            for pr in range(NH // 2):
                hA = 2 * pr
                # transpose q and k for the pair: [P, 2, 64] -> [128, P]
                qT = qkpool.tile([P, MT, P], F32R, name="qT", tag="qT")
                kT = qkpool.tile([P, MT, P], F32R, name="kT", tag="kT")
                # transpose outputs must land at PSUM partition 0; head B's
                # half is moved to partitions 64-127 by a PSUM->SBUF DMA.
                for dst, c0 in ((qT, 0), (kT, D)):
                    for hh in range(2):
                        for mg in range(2):
                            tp = pw.tile([D, 512], F32, name="tp", tag="pw")
                            for mi in range(4):
                                m = 4 * mg + mi
                                nc.tensor.transpose(
                                    tp[:, ts(mi, P)],
                                    h_sb[:, m, hA + hh, c0 : c0 + D].bitcast(F32),
                                    identity,
                                )
                            dslice = dst[hh * D : hh * D + D, 4 * mg : 4 * mg + 4, :]
                            if hh == 0:
                                nc.vector.tensor_copy(dslice, tp)
                            else:
                                stg = zpool.tile([D, 512], F32R, name="stg", tag="stg", bufs=2)
                                nc.vector.tensor_copy(stg, tp)
                                nc.sync.dma_start(out=dslice, in_=stg)

                for hh in range(2):
                    h = hA + hh
                    base = hh * D
                    qTh = qT[base : base + D].rearrange("p a b -> p (a b)")
                    et = etpool.tile([P, MT, S], F32R, name="et", tag="et")
                    for jt in range(MT):
                        st = pst.tile([P, S], F32, name="st", tag="st")
                        for n2 in range(2):
                            nc.tensor.matmul(
                                st[:, ts(n2, 512)],
                                lhsT=kT[base : base + D, jt, :],
                                rhs=qTh[:, ts(n2, 512)],
                                start=True,
                                stop=True,
                            )
                        nc.scalar.activation(out=et[:, jt, :], in_=st, func=Act.Exp)
                    # PV with fused Z (row 64): lhsT = [v'(64) | ones] = h cols 128..193
                    res_ps = pres.tile([D + 1, S], F32, name="res_ps", tag="res")
                    for jt in range(MT):
                        for n2 in range(2):
                            nc.tensor.matmul(
                                res_ps[:, ts(n2, 512)],
                                lhsT=h_sb[:, jt, h, 128:193],
                                rhs=et[:, jt, ts(n2, 512)],
                                start=(jt == 0),
                                stop=(jt == MT - 1),
                            )
                    # normalize: recip Z, broadcast to 64 partitions, multiply
                    zrec = zpool.tile([P, S], F32, name="zrec", tag="zrec")
                    nc.vector.reciprocal(out=zrec[D : D + 1, :], in_=res_ps[D : D + 1, :])
                    # broadcast the Z-recip row to 64 partitions via a DRAM
                    # round-trip (SBUF APs cannot have zero partition step).
                    zdram = dpool.tile([1, S], F32, name="zdram", tag="zdram")
                    nc.sync.dma_start(out=zdram, in_=zrec[D : D + 1, :])
                    zb = zpool.tile([D, S], F32, name="zb", tag="zb")
                    nc.sync.dma_start(
                        out=zb, in_=zdram[0:1, :].partition_broadcast(D)
                    )
                    # DVE reads/writes the same physical lanes, so odd heads
                    # (partitions 64-127 of resn) go through a DMA shift.
                    if base == 0:
                        nc.vector.tensor_mul(
                            out=resn[0:D, pr, :], in0=res_ps[0:D, :], in1=zb
                        )
                    else:
                        nc.vector.tensor_mul(out=zb, in0=res_ps[0:D, :], in1=zb)
                        nc.sync.dma_start(
                            out=resn[base : base + D, pr, :], in_=zb.bitcast(F32R)
                        )

            # ---- output projection + residual ----
            for mo in range(KT):
                for n2 in range(2):
                    ops = pw.tile([P, 512], F32, name="ops", tag="pw")
                    for k in range(KT):
                        nc.tensor.matmul(
                            ops,
                            lhsT=wo_sb[:, k, ts(mo, P)],
                            rhs=resn[:, k, ts(n2, 512)],
                            start=(k == 0),
                            stop=(k == KT - 1),
                        )
                    yt = ypool.tile([P, 512], F32, name="yt", tag="yt")
                    nc.vector.tensor_add(
                        out=yt, in0=ops, in1=xb[:, mo, ts(n2, 512)].bitcast(F32)
                    )
                    nc.sync.dma_start(
                        out=y[b].rearrange("(kt p) s -> p kt s", p=P)[:, mo, ts(n2, 512)],
                        in_=yt,
                    )
    return nc


_NC_CACHE = {}


def _get_nc():
    if "nc" not in _NC_CACHE:
        nc = bacc.Bacc(trn_type="TRN2")
        build_kernel(nc)
        if not nc.is_finalized():
            nc.finalize()
        _NC_CACHE["nc"] = nc
    return _NC_CACHE["nc"]


def kernel(x, w_proj, w_out):
    """x [16,512,32,32] f32, w_proj [1536,512], w_out [512,512] -> [16,512,32,32]."""
    from concourse.bass_utils import run_bass_kernel_spmd

    n_cores = 8
    B = x.shape[0]
    xs = np.ascontiguousarray(x.reshape(B, C, S), dtype=np.float32)
    wpT_np = np.ascontiguousarray(w_proj.T, dtype=np.float32)
    woT_np = np.ascontiguousarray(w_out.T, dtype=np.float32)

    nc = _get_nc()
    in_maps = [
        {
            "x": np.ascontiguousarray(xs[B_LOCAL * c : B_LOCAL * (c + 1)]),
            "wpT": wpT_np,
            "woT": woT_np,
        }
        for c in range(n_cores)
    ]
    res = run_bass_kernel_spmd(nc, in_maps, core_ids=list(range(n_cores)))
    y = np.concatenate([rr["y"] for rr in res.results], axis=0)
    return np.ascontiguousarray(y.reshape(B, C, 32, 32))
